# revision 36
# baseline (speedup 1.0000x reference)
"""MultiHeadAttention (Enformer-style relative-position attention) on 8 trn2 cores.

Sharding: core c handles batch b = c//4 and heads {2g, 2g+1} with g = c%4.
Attention is computed per (batch, head). The per-head attention outputs are
exchanged with a 4-way AllToAll (bf16) so each core ends up with all 8 heads'
outputs for its 384 sequence rows, then computes the final embedding
projection locally: core c produces output rows [384g, 384(g+1)) of batch b.

relative_shift: the per-i-tile rel-logit band [128, 1663] is written to DRAM
contiguously and read back with a skewed access pattern (row p starts at
offset 127 - p), which is a regular strided DMA.

attn transpose: attn rows [128, 1536] are written to DRAM and read back with
one batched xbar-transpose DMA per i-tile (dest [128, 12, 128]), which avoids
the per-128x128-block SBUF->SBUF transpose storm on the sync engine.

x is transposed on the host so no on-device transposes are needed for the
QKV projections.
"""
import math
import numpy as np

import concourse.bass as bass
from concourse import bacc
import concourse.mybir as mybir
import concourse.tile as tile
from concourse.bass_utils import run_bass_kernel_spmd

# problem shapes (hardcoded per contract)
B, L, D = 2, 1536, 1536
H, K, V, F = 8, 64, 192, 192
P = 128
NCORES = 8
HPC = 2              # heads per core
LS = L // 4          # 384: L-slice per core in the final output
NKT = D // P         # 12 contraction tiles
NIT = L // P         # 12 i-tiles
PE_LEN = 2 * L - 1   # 3071
PE_PAD = 2 * L       # 3072: padded so fp32r matmul widths are even
BAND = L + P - 1     # 1663 logical band width per i-tile
BANDW = L + P        # 1664: stored band row pitch (even matmul widths)
BCH = [416, 416, 416, 416]   # band chunks (psum bank <= 512 fp32, even)
CH = 512             # L-chunk
NCH = L // CH        # 3
M96 = 96             # partition height of outT tiles (2 per head)
SH = LS * LS         # 147456: A2A shard elements

F32 = mybir.dt.float32
F32R = mybir.dt.float32r
BF16 = mybir.dt.bfloat16
LN2 = float(np.log(2.0))


# ----------------------------------------------------------------------------
# host-side constants: positional features (input-independent)
# ----------------------------------------------------------------------------

def _positional_features() -> np.ndarray:
    """Replicates reference.positional_features_all(arange(-L+1, L), F, L)."""
    pos = np.arange(-L + 1, L, dtype=np.float64)
    x = np.abs(pos)[:, None]                      # [3071, 1]
    f = F // 6                                    # 32

    # exponential
    max_half_life = np.log(L) / np.log(2.0)
    half_life = 2.0 ** np.linspace(3.0, max_half_life, f)
    feat_exp = np.exp(-LN2 / half_life[None, :] * x)

    # central mask
    widths = 2.0 ** np.arange(1, f + 1, dtype=np.float64) - 1.0
    feat_cm = (widths[None, :] > x).astype(np.float64)

    # gamma
    stddev = L / (2.0 * f)
    start_mean = L / f
    mean = np.linspace(start_mean, float(L), f)
    concentration = (mean / stddev) ** 2
    rate = mean / (stddev ** 2)
    safe_x = np.maximum(x, 1e-300)
    log_unnorm = (concentration[None, :] - 1.0) * np.log(safe_x) - rate[None, :] * x
    # xlogy(a, 0): 0 if a == 0 else -inf
    zero_x = x == 0.0
    conc_one = np.isclose(concentration[None, :] - 1.0, 0.0)
    log_unnorm = np.where(zero_x & ~conc_one, -np.inf, log_unnorm)
    log_unnorm = np.where(zero_x & conc_one, -rate[None, :] * x, log_unnorm)
    lgamma = np.vectorize(math.lgamma)
    log_norm = lgamma(concentration) - concentration * np.log(rate)
    p = np.exp(log_unnorm - log_norm[None, :]) + 1e-8
    feat_gamma = p / p.max()

    emb = np.concatenate([feat_exp, feat_cm, feat_gamma], axis=-1)   # [3071, 96]
    sign = np.sign(pos)[:, None]
    emb = np.concatenate([emb, sign * emb], axis=-1)                 # [3071, 192]
    return emb.astype(np.float32)


# ----------------------------------------------------------------------------
# device program
# ----------------------------------------------------------------------------

def _ap(t, offset, dims):
    return bass.AP(t.tensor if hasattr(t, "tensor") else t, offset, dims)


DEBUG = False


def _build_nc():
    nc = bacc.Bacc("TRN2", num_devices=NCORES, target_bir_lowering=False)

    xt_in = nc.dram_tensor("xt", [D, L], F32R, kind="ExternalInput")
    wqk_in = nc.dram_tensor("wqk", [D, 2 * P], F32R, kind="ExternalInput")
    wv_in = nc.dram_tensor("wv", [D, HPC * V], F32R, kind="ExternalInput")
    wrel_in = nc.dram_tensor("wrel", [2 * P, P], F32R, kind="ExternalInput")
    pet_in = nc.dram_tensor("pet", [2 * P, PE_PAD], F32R, kind="ExternalInput")
    wemb_in = nc.dram_tensor("wemb", [H * V, D], BF16, kind="ExternalInput")
    qbias_in = nc.dram_tensor("qbias", [P, 2], F32, kind="ExternalInput")
    bemb_in = nc.dram_tensor("bemb", [1, D], F32, kind="ExternalInput")
    coff_in = nc.dram_tensor("coff", [1, 1], mybir.dt.uint32, kind="ExternalInput")
    out_t = nc.dram_tensor("out", [LS, D], F32, kind="ExternalOutput")

    # Per-head AllGathers: each core contributes a [192 vc, 1536 m] bf16 outT
    # slab per head; head-0's collective overlaps head-1's attention compute.
    # ccout1 = heads {0,2,4,6}, ccout2 = heads {1,3,5,7} (W_emb rows are
    # reordered on the host to match).
    ccin1 = nc.dram_tensor("ccin1", [V * L], BF16)
    ccin2 = nc.dram_tensor("ccin2", [V * L], BF16)
    ccout1 = nc.dram_tensor("ccout1", [4 * V * L], BF16)
    ccout2 = nc.dram_tensor("ccout2", [4 * V * L], BF16)

    if DEBUG:
        dbg_qcT = nc.dram_tensor("dbg_qcT", [P, L], F32, kind="ExternalOutput")
        dbg_kT = nc.dram_tensor("dbg_kT", [P, L], F32, kind="ExternalOutput")
        dbg_rkT = nc.dram_tensor("dbg_rkT", [P, PE_PAD], F32, kind="ExternalOutput")
        dbg_attn = nc.dram_tensor("dbg_attn", [P, L], BF16, kind="ExternalOutput")
        dbg_sums = nc.dram_tensor("dbg_sums", [P, NIT], F32, kind="ExternalOutput")
        dbg_attnT = nc.dram_tensor("dbg_attnT", [P, NIT * L], BF16, kind="ExternalOutput")
        dbg_outT = nc.dram_tensor("dbg_outT", [M96, L], BF16, kind="ExternalOutput")
        dbg_oall = nc.dram_tensor("dbg_oall", [P, NKT * LS], BF16, kind="ExternalOutput")

    rg = [[0, 1, 2, 3], [4, 5, 6, 7]]

    with tile.TileContext(nc) as tc:
        with (
            tc.tile_pool(name="consts", bufs=1) as consts,
            tc.tile_pool(name="proj", bufs=1) as proj,
            tc.tile_pool(name="dram", bufs=6, space="DRAM") as dpool,
            tc.tile_pool(name="adram", bufs=6, space="DRAM") as adpool,
        ):
            ident = consts.tile([P, P], BF16)
            from concourse.masks import make_identity
            make_identity(nc, ident[:])

            qbias = consts.tile([P, 2], F32)
            nc.sync.dma_start(qbias[:], qbias_in[:, :])
            bemb = consts.tile([P, D], F32)
            nc.sync.dma_start(bemb[:], _ap(bemb_in, 0, [[0, P], [1, D]]))

            # persistent data produced by projections
            qcT = proj.tile([P, L], F32R)   # (q*scale + rcb)^T, heads stacked
            qpT = proj.tile([P, L], F32R)   # (q*scale + rpb)^T
            kT = proj.tile([P, L], F32R)    # k^T, heads stacked
            vsb = proj.tile([P, NIT, HPC * V], BF16)   # v, j on partitions
            rkT = proj.tile([P, PE_PAD], F32R)         # rel_k^T, heads stacked

            # ---------------- phase D: rel_k ----------------
            with (
                tc.tile_pool(name="dw", bufs=1) as dw,
                tc.tile_pool(name="d_ps", bufs=2, space="PSUM") as d_ps,
            ):
                wrel = dw.tile([P, 2, P], F32R)
                pet = dw.tile([P, 2, PE_PAD], F32R)
                nc.scalar.dma_start(
                    wrel[:], _ap(wrel_in, 0, [[P, P], [P * P, 2], [1, P]])
                )
                nc.scalar.dma_start(
                    pet[:], _ap(pet_in, 0, [[PE_PAD, P], [P * PE_PAD, 2], [1, PE_PAD]])
                )
                for nj in range(6):
                    ps = d_ps.tile([P, 512], F32)
                    for kt in range(2):
                        nc.tensor.matmul(
                            ps[:],
                            wrel[:, kt, :],
                            pet[:, kt, nj * 512:(nj + 1) * 512],
                            start=(kt == 0), stop=(kt == 1),
                        )
                    nc.vector.tensor_copy(rkT[:, nj * 512:(nj + 1) * 512], ps[:])

            # ---------------- phases B/C: projections per L-chunk ----------------
            with (
                tc.tile_pool(name="w_qkv", bufs=1) as w_qkv,
                tc.tile_pool(name="xt_pool", bufs=2) as xt_pool,
                tc.tile_pool(name="b_ps", bufs=2, space="PSUM") as b_ps,
                tc.tile_pool(name="c_ps", bufs=3, space="PSUM") as c_ps,
            ):
                wqk = w_qkv.tile([P, NKT, 2 * P], F32R)
                wv = w_qkv.tile([P, NKT, HPC * V], F32R)
                nc.scalar.dma_start(
                    wqk[:], _ap(wqk_in, 0, [[2 * P, P], [P * 2 * P, NKT], [1, 2 * P]])
                )
                nc.scalar.dma_start(
                    wv[:],
                    _ap(wv_in, 0, [[HPC * V, P], [P * HPC * V, NKT], [1, HPC * V]]),
                )

                for lc in range(NCH):
                    xt = xt_pool.tile([P, NKT, CH], F32R, tag="xt")
                    # one DMA: x^T slice [D, CH] -> [128, 12, CH]
                    nc.sync.dma_start(
                        xt[:],
                        _ap(xt_in, lc * CH, [[L, P], [P * L, NKT], [1, CH]]),
                    )
                    # B: q/k projections for this chunk
                    for mi in range(2):
                        ps = b_ps.tile([P, CH], F32)
                        for kt in range(NKT):
                            nc.tensor.matmul(
                                ps[:],
                                wqk[:, kt, mi * P:(mi + 1) * P],
                                xt[:, kt, :],
                                start=(kt == 0), stop=(kt == NKT - 1),
                            )
                        sl = slice(lc * CH, (lc + 1) * CH)
                        if mi == 0:
                            nc.scalar.activation(
                                qcT[:, sl], ps[:], mybir.ActivationFunctionType.Identity,
                                bias=qbias[:, 0:1], scale=float(K) ** -0.5,
                            )
                            nc.scalar.activation(
                                qpT[:, sl], ps[:], mybir.ActivationFunctionType.Identity,
                                bias=qbias[:, 1:2], scale=float(K) ** -0.5,
                            )
                        else:
                            nc.vector.tensor_copy(kT[:, sl], ps[:])
                    # C: v projection for this chunk
                    for j4 in range(CH // P):
                        ps = c_ps.tile([P, HPC * V], F32)
                        for kt in range(NKT):
                            nc.tensor.matmul(
                                ps[:],
                                xt[:, kt, j4 * P:(j4 + 1) * P],
                                wv[:, kt, :],
                                start=(kt == 0), stop=(kt == NKT - 1),
                            )
                        nc.vector.tensor_copy(vsb[:, lc * (CH // P) + j4, :], ps[:])

            if DEBUG:
                nc.sync.dma_start(dbg_qcT[:, :], qcT[:].bitcast(F32))
                nc.sync.dma_start(dbg_kT[:, :], kT[:].bitcast(F32))
                nc.sync.dma_start(dbg_rkT[:, :], rkT[:].bitcast(F32))

            # ---------------- phase E: attention ----------------
            with (
                tc.tile_pool(name="attnT_p", bufs=2) as attnT_p,
                tc.tile_pool(name="outT_p", bufs=2) as outT_p,
                tc.tile_pool(name="sums_p", bufs=2) as sums_p,
                tc.tile_pool(name="band_sb_p", bufs=4) as band_sb_p,
                tc.tile_pool(name="rel_p", bufs=4) as rel_p,
                tc.tile_pool(name="attn_p", bufs=4) as attn_p,
                tc.tile_pool(name="band_ps", bufs=1, space="PSUM") as band_ps,
                tc.tile_pool(name="cont_ps", bufs=2, space="PSUM") as cont_ps,
                tc.tile_pool(name="o_ps", bufs=1, space="PSUM") as o_ps,
            ):
                for h in range(HPC):
                    hp = slice(h * K, (h + 1) * K)   # partition slice of this head
                    attnT = attnT_p.tile([P, NIT, L], BF16, tag="attnT")
                    sums = sums_p.tile([P, NIT], F32, tag="sums")

                    for it in range(NIT):
                        p0 = L - P - it * P   # band start: 1408 - 128*it
                        # rel band matmul -> bf16 band in sbuf
                        band_sb = band_sb_p.tile([P, BANDW], BF16, tag="band")
                        off = 0
                        for ci, cw in enumerate(BCH):
                            ps = band_ps.tile([P, 512], F32, tag="band_ps")
                            nc.tensor.matmul(
                                ps[:, :cw],
                                qpT[hp, it * P:(it + 1) * P],
                                rkT[hp, p0 + off:p0 + off + cw],
                                start=True, stop=True,
                            )
                            if ci % 2 == 0:
                                nc.vector.tensor_copy(
                                    band_sb[:, off:off + cw], ps[:, :cw]
                                )
                            else:
                                nc.scalar.copy(band_sb[:, off:off + cw], ps[:, :cw])
                            off += cw
                        band_dram = dpool.tile([P * BANDW], BF16, tag="band_dram")
                        # same HWDGE ring (ACT) as the skewed read below: ring
                        # FIFO guarantees the write drains before the read.
                        nc.scalar.dma_start(
                            band_dram.rearrange("(p w) -> p w", p=P), band_sb[:]
                        )

                        # content logits
                        pc = cont_ps.tile([P, L], F32, tag="cont")
                        for nj in range(NCH):
                            nc.tensor.matmul(
                                pc[:, nj * CH:(nj + 1) * CH],
                                qcT[hp, it * P:(it + 1) * P],
                                kT[hp, nj * CH:(nj + 1) * CH],
                                start=True, stop=False,
                            )

                        # shifted rel read-back: rel[p, j] = band[p, j + 127 - p]
                        rel_sb = rel_p.tile([P, L], BF16, tag="rel")
                        diag = _ap(
                            band_dram.tensor,
                            band_dram.offset + (P - 1),
                            [[BANDW - 1, P], [1, L]],
                        )
                        nc.scalar.dma_start(rel_sb[:], diag)
                        # add rel into pc on the PE: pc[:, c] += I @ rel[:, c]
                        for nj in range(NCH):
                            nc.tensor.matmul(
                                pc[:, nj * CH:(nj + 1) * CH],
                                ident[:],
                                rel_sb[:, nj * CH:(nj + 1) * CH],
                                start=False, stop=True,
                            )

                        # exp + row sums; bf16 attn
                        attn_sb = attn_p.tile([P, L], BF16, tag="attn")
                        nc.scalar.activation(
                            attn_sb[:], pc[:], mybir.ActivationFunctionType.Exp,
                            accum_out=sums[:, it:it + 1],
                        )
                        # attn -> DRAM -> batched transposed read into attnT
                        # (write + transposed read on the same SP HWDGE ring)
                        attn_dram = adpool.tile([P * L], BF16, tag="attn_dram")
                        nc.sync.dma_start(
                            attn_dram.rearrange("(p w) -> p w", p=P), attn_sb[:]
                        )
                        nc.sync.dma_start(
                            attnT[:, :, it * P:(it + 1) * P],
                            _ap(attn_dram.tensor, attn_dram.offset, [[L, P], [1, L]]),
                            transpose=True,
                        )
                        if DEBUG and h == 0 and it == 0:
                            nc.sync.dma_start(dbg_attn[:, :], attn_sb[:])

                    if DEBUG and h == 0:
                        nc.sync.dma_start(
                            dbg_attnT[:, :],
                            attnT.rearrange("p a b -> p (a b)"),
                        )

                    # reciprocal of row sums -> broadcast [M96, L] via DRAM
                    recip = sums_p.tile([P, NIT], F32, tag="recip")
                    nc.vector.reciprocal(recip[:], sums[:])
                    # recip roundtrip on the otherwise-idle SWDGE (gpsimd) ring
                    # so it does not queue behind band/attn traffic; write and
                    # read stay on the same ring for ordering.
                    recip_dram = dpool.tile([L], F32, tag="recip_dram")
                    nc.gpsimd.dma_start(
                        recip_dram.rearrange("(it p) -> p it", p=P), recip[:]
                    )
                    recip_bc = sums_p.tile([M96, L], F32, tag="recip_bc")
                    nc.gpsimd.dma_start(
                        recip_bc[:],
                        _ap(recip_dram.tensor, recip_dram.offset, [[0, M96], [1, L]]),
                    )

                    # outT = v^T @ attnT, normalized; 2 tiles of 96 rows per head
                    for sub in range(2):
                        vs = h * V + sub * M96
                        outT_sb = outT_p.tile([M96, L], BF16, tag="outT_sb")
                        for ni in range(NCH):
                            po = o_ps.tile([P, CH], F32, tag="o_ps")
                            for jt in range(NIT):
                                nc.tensor.matmul(
                                    po[0:M96, :],
                                    vsb[:, jt, vs:vs + M96],
                                    attnT[:, jt, ni * CH:(ni + 1) * CH],
                                    start=(jt == 0), stop=(jt == NIT - 1),
                                )
                            nc.vector.tensor_tensor(
                                outT_sb[:, ni * CH:(ni + 1) * CH],
                                po[0:M96, :],
                                recip_bc[:, ni * CH:(ni + 1) * CH],
                                mybir.AluOpType.mult,
                            )
                        # store into this head's ccin: [vc 96, 1536 m]; on the
                        # gpsimd ring so the AllGather is not queued behind
                        # band/attn DMA traffic (the collective waits on the
                        # writer's completion semaphore, not the ring).
                        vcb = sub * M96 * L
                        ccin_h = ccin1 if h == 0 else ccin2
                        nc.gpsimd.dma_start(
                            _ap(ccin_h, vcb, [[L, M96], [1, L]]),
                            outT_sb[:],
                        )
                        if DEBUG and h == 0 and sub == 0:
                            nc.sync.dma_start(dbg_outT[:, :], outT_sb[:])
                    if DEBUG and h == 0:
                        nc.sync.dma_start(dbg_sums[:, :], sums[:])

                    # AllGather this head's slab; head 0's collective runs
                    # while head 1's attention is still computing.
                    nc.gpsimd.collective_compute(
                        "AllGather",
                        mybir.AluOpType.bypass,
                        replica_groups=rg,
                        ins=[(ccin1 if h == 0 else ccin2)[:]],
                        outs=[(ccout1 if h == 0 else ccout2)[:]],
                    )

            with (
                tc.tile_pool(name="fin_sb", bufs=1) as fin_sb,
                tc.tile_pool(name="fout_p", bufs=3) as fout_p,
                tc.tile_pool(name="f_ps", bufs=2, space="PSUM") as f_ps,
            ):
                # full W_emb (bf16) for the post-AG projection: [128, 12, D];
                # loaded here so its SBUF does not occupy the attention phase
                # and the DMA overlaps the AllGather.
                wemb = fin_sb.tile([P, NKT, D], BF16)
                nc.scalar.dma_start(
                    wemb[:], _ap(wemb_in, 0, [[D, P], [P * D, NKT], [1, D]])
                )
                # keep the PE warm (HAM at 8/8) through the AllGather window;
                # results are never read.
                dummy_ps = f_ps.tile([P, CH], F32, tag="dummy_ps")
                for w in range(80):
                    nc.tensor.matmul(
                        dummy_ps[:],
                        qpT[0:K, 0:P],
                        rkT[0:K, (w % 6) * CH:(w % 6 + 1) * CH],
                        start=True, stop=True,
                    )

                # ccout{1,2}: [768 vc, 1536 m] each; read this core's m-slice
                # (dynamic offset coff = (c%4)*384 elements) -> [128, 6, 384]
                tmp = nc.sync.alloc_register("coff_reg")
                nc.sync.reg_load(tmp, coff_in[0:1, 0:1])
                coff = nc.sync.snap(tmp, donate=True, min_val=0, max_val=3 * LS)
                oall1 = fin_sb.tile([P, NKT // 2, LS], BF16)
                nc.sync.dma_start(
                    oall1[:], _ap(ccout1, coff, [[L, P], [P * L, NKT // 2], [1, LS]])
                )
                oall2 = fin_sb.tile([P, NKT // 2, LS], BF16)
                nc.sync.dma_start(
                    oall2[:], _ap(ccout2, coff, [[L, P], [P * L, NKT // 2], [1, LS]])
                )
                for mi in range(LS // P):
                    for nj in range(NCH):
                        pf = f_ps.tile([P, CH], F32, tag="f_ps")
                        for kt in range(NKT):
                            src = oall1 if kt < NKT // 2 else oall2
                            nc.tensor.matmul(
                                pf[:],
                                src[:, kt % (NKT // 2), mi * P:(mi + 1) * P],
                                wemb[:, kt, nj * CH:(nj + 1) * CH],
                                start=(kt == 0), stop=(kt == NKT - 1),
                            )
                        fo = fout_p.tile([P, CH], F32, tag="fout")
                        nc.vector.tensor_tensor(
                            fo[:], pf[:], bemb[:, nj * CH:(nj + 1) * CH],
                            mybir.AluOpType.add,
                        )
                        nc.sync.dma_start(
                            out_t[mi * P:(mi + 1) * P, nj * CH:(nj + 1) * CH], fo[:]
                        )

    nc.compile()
    return nc


_CACHE = {}


def _get_nc():
    if "nc" not in _CACHE:
        _CACHE["nc"] = _build_nc()
    return _CACHE["nc"]


def _make_in_maps(inputs, Wq, Wk, Wv, W_rel, W_emb, b_emb, rcb, rpb):
    import ml_dtypes

    pe = _positional_features()          # [3071, 192]
    pet = np.zeros((2 * P, PE_PAD), np.float32)
    pet[:F, :PE_LEN] = pe.T

    Wq_h = Wq.reshape(D, H, K)
    Wk_h = Wk.reshape(D, H, K)
    Wv_h = Wv.reshape(D, H, V)
    Wrel_h = W_rel.reshape(F, H, K)
    # rows reordered to match the two per-head AllGather outputs:
    # ccout1 = heads {0,2,4,6}, ccout2 = heads {1,3,5,7}
    wemb_perm = W_emb.reshape(H, V, D)[[0, 2, 4, 6, 1, 3, 5, 7]].reshape(H * V, D)
    wemb_bf = np.ascontiguousarray(wemb_perm).astype(ml_dtypes.bfloat16)

    in_maps = []
    for c in range(NCORES):
        b = c // 4
        g = c % 4
        h0, h1 = 2 * g, 2 * g + 1
        wqk = np.concatenate(
            [Wq_h[:, h0], Wq_h[:, h1], Wk_h[:, h0], Wk_h[:, h1]], axis=1
        )  # [D, 256]: Q stacked then K stacked
        wv2 = np.concatenate([Wv_h[:, h0], Wv_h[:, h1]], axis=1)  # [D, 384]
        wrel = np.zeros((2 * P, P), np.float32)
        wrel[:F, :K] = Wrel_h[:, h0]
        wrel[:F, K:] = Wrel_h[:, h1]
        qbias = np.stack(
            [np.concatenate([rcb[h0], rcb[h1]]), np.concatenate([rpb[h0], rpb[h1]])],
            axis=1,
        )  # [128, 2]
        in_maps.append({
            "xt": np.ascontiguousarray(inputs[b].T),
            "wqk": np.ascontiguousarray(wqk),
            "wv": np.ascontiguousarray(wv2),
            "wrel": wrel,
            "pet": pet,
            "wemb": wemb_bf,
            "qbias": np.ascontiguousarray(qbias),
            "bemb": b_emb.reshape(1, D),
            "coff": np.array([[g * LS]], dtype=np.uint32),
        })
    return in_maps


# ----------------------------------------------------------------------------
# entry point
# ----------------------------------------------------------------------------

def kernel(inputs, Wq, Wk, Wv, W_rel, W_emb, b_emb, rel_content_bias, rel_pos_bias):
    inputs = np.asarray(inputs, np.float32)
    Wq = np.asarray(Wq, np.float32)
    Wk = np.asarray(Wk, np.float32)
    Wv = np.asarray(Wv, np.float32)
    W_rel = np.asarray(W_rel, np.float32)
    W_emb = np.asarray(W_emb, np.float32)
    b_emb = np.asarray(b_emb, np.float32)
    rcb = np.asarray(rel_content_bias, np.float32).reshape(H, K)
    rpb = np.asarray(rel_pos_bias, np.float32).reshape(H, K)

    in_maps = _make_in_maps(inputs, Wq, Wk, Wv, W_rel, W_emb, b_emb, rcb, rpb)
    nc = _get_nc()
    res = run_bass_kernel_spmd(nc, in_maps, core_ids=list(range(NCORES)))

    out = np.empty((B, L, D), np.float32)
    for c in range(NCORES):
        b = c // 4
        g = c % 4
        out[b, g * LS:(g + 1) * LS, :] = res.results[c]["out"]
    return out


# revision 38
# speedup vs baseline: 1.0248x; 1.0248x over previous
"""MultiHeadAttention (Enformer-style relative-position attention) on 8 trn2 cores.

Sharding: core c handles batch b = c//4 and heads {2g, 2g+1} with g = c%4.
Attention is computed per (batch, head). The per-head attention outputs are
exchanged with a 4-way AllToAll (bf16) so each core ends up with all 8 heads'
outputs for its 384 sequence rows, then computes the final embedding
projection locally: core c produces output rows [384g, 384(g+1)) of batch b.

relative_shift: the per-i-tile rel-logit band [128, 1663] is written to DRAM
contiguously and read back with a skewed access pattern (row p starts at
offset 127 - p), which is a regular strided DMA.

attn transpose: attn rows [128, 1536] are written to DRAM and read back with
one batched xbar-transpose DMA per i-tile (dest [128, 12, 128]), which avoids
the per-128x128-block SBUF->SBUF transpose storm on the sync engine.

x is transposed on the host so no on-device transposes are needed for the
QKV projections.
"""
import math
import numpy as np

import concourse.bass as bass
from concourse import bacc
import concourse.mybir as mybir
import concourse.tile as tile
from concourse.bass_utils import run_bass_kernel_spmd

# problem shapes (hardcoded per contract)
B, L, D = 2, 1536, 1536
H, K, V, F = 8, 64, 192, 192
P = 128
NCORES = 8
HPC = 2              # heads per core
LS = L // 4          # 384: L-slice per core in the final output
NKT = D // P         # 12 contraction tiles
NIT = L // P         # 12 i-tiles
PE_LEN = 2 * L - 1   # 3071
PE_PAD = 2 * L       # 3072: padded so fp32r matmul widths are even
BAND = L + P - 1     # 1663 logical band width per i-tile
BANDW = L + P        # 1664: stored band row pitch (even matmul widths)
BCH = [416, 416, 416, 416]   # band chunks (psum bank <= 512 fp32, even)
CH = 512             # L-chunk
NCH = L // CH        # 3
M96 = 96             # partition height of outT tiles (2 per head)
SH = LS * LS         # 147456: A2A shard elements

F32 = mybir.dt.float32
F32R = mybir.dt.float32r
BF16 = mybir.dt.bfloat16
LN2 = float(np.log(2.0))


# ----------------------------------------------------------------------------
# host-side constants: positional features (input-independent)
# ----------------------------------------------------------------------------

def _positional_features() -> np.ndarray:
    """Replicates reference.positional_features_all(arange(-L+1, L), F, L)."""
    pos = np.arange(-L + 1, L, dtype=np.float64)
    x = np.abs(pos)[:, None]                      # [3071, 1]
    f = F // 6                                    # 32

    # exponential
    max_half_life = np.log(L) / np.log(2.0)
    half_life = 2.0 ** np.linspace(3.0, max_half_life, f)
    feat_exp = np.exp(-LN2 / half_life[None, :] * x)

    # central mask
    widths = 2.0 ** np.arange(1, f + 1, dtype=np.float64) - 1.0
    feat_cm = (widths[None, :] > x).astype(np.float64)

    # gamma
    stddev = L / (2.0 * f)
    start_mean = L / f
    mean = np.linspace(start_mean, float(L), f)
    concentration = (mean / stddev) ** 2
    rate = mean / (stddev ** 2)
    safe_x = np.maximum(x, 1e-300)
    log_unnorm = (concentration[None, :] - 1.0) * np.log(safe_x) - rate[None, :] * x
    # xlogy(a, 0): 0 if a == 0 else -inf
    zero_x = x == 0.0
    conc_one = np.isclose(concentration[None, :] - 1.0, 0.0)
    log_unnorm = np.where(zero_x & ~conc_one, -np.inf, log_unnorm)
    log_unnorm = np.where(zero_x & conc_one, -rate[None, :] * x, log_unnorm)
    lgamma = np.vectorize(math.lgamma)
    log_norm = lgamma(concentration) - concentration * np.log(rate)
    p = np.exp(log_unnorm - log_norm[None, :]) + 1e-8
    feat_gamma = p / p.max()

    emb = np.concatenate([feat_exp, feat_cm, feat_gamma], axis=-1)   # [3071, 96]
    sign = np.sign(pos)[:, None]
    emb = np.concatenate([emb, sign * emb], axis=-1)                 # [3071, 192]
    return emb.astype(np.float32)


# ----------------------------------------------------------------------------
# device program
# ----------------------------------------------------------------------------

def _ap(t, offset, dims):
    return bass.AP(t.tensor if hasattr(t, "tensor") else t, offset, dims)


DEBUG = False


def _build_nc():
    nc = bacc.Bacc("TRN2", num_devices=NCORES, target_bir_lowering=False)

    xt_in = nc.dram_tensor("xt", [D, L], F32R, kind="ExternalInput")
    wqk_in = nc.dram_tensor("wqk", [D, 2 * P], F32R, kind="ExternalInput")
    wv_in = nc.dram_tensor("wv", [D, HPC * V], F32R, kind="ExternalInput")
    wrel_in = nc.dram_tensor("wrel", [2 * P, P], F32R, kind="ExternalInput")
    pet_in = nc.dram_tensor("pet", [2 * P, PE_PAD], F32R, kind="ExternalInput")
    wemb_in = nc.dram_tensor("wemb", [H * V, D], BF16, kind="ExternalInput")
    qbias_in = nc.dram_tensor("qbias", [P, 2], F32, kind="ExternalInput")
    bemb_in = nc.dram_tensor("bemb", [1, D], F32, kind="ExternalInput")
    coff_in = nc.dram_tensor("coff", [1, 1], mybir.dt.uint32, kind="ExternalInput")
    out_t = nc.dram_tensor("out", [LS, D], F32, kind="ExternalOutput")

    # Per-head AllGathers: each core contributes a [192 vc, 1536 m] bf16 outT
    # slab per head; head-0's collective overlaps head-1's attention compute.
    # ccout1 = heads {0,2,4,6}, ccout2 = heads {1,3,5,7} (W_emb rows are
    # reordered on the host to match).
    ccin1 = nc.dram_tensor("ccin1", [V * L], BF16)
    ccin2 = nc.dram_tensor("ccin2", [V * L], BF16)
    ccout1 = nc.dram_tensor("ccout1", [4 * V * L], BF16)
    ccout2 = nc.dram_tensor("ccout2", [4 * V * L], BF16)

    if DEBUG:
        dbg_qcT = nc.dram_tensor("dbg_qcT", [P, L], F32, kind="ExternalOutput")
        dbg_kT = nc.dram_tensor("dbg_kT", [P, L], F32, kind="ExternalOutput")
        dbg_rkT = nc.dram_tensor("dbg_rkT", [P, PE_PAD], F32, kind="ExternalOutput")
        dbg_attn = nc.dram_tensor("dbg_attn", [P, L], BF16, kind="ExternalOutput")
        dbg_sums = nc.dram_tensor("dbg_sums", [P, NIT], F32, kind="ExternalOutput")
        dbg_attnT = nc.dram_tensor("dbg_attnT", [P, NIT * L], BF16, kind="ExternalOutput")
        dbg_outT = nc.dram_tensor("dbg_outT", [M96, L], BF16, kind="ExternalOutput")
        dbg_oall = nc.dram_tensor("dbg_oall", [P, NKT * LS], BF16, kind="ExternalOutput")

    rg = [[0, 1, 2, 3], [4, 5, 6, 7]]

    with tile.TileContext(nc) as tc:
        with (
            tc.tile_pool(name="consts", bufs=1) as consts,
            tc.tile_pool(name="proj", bufs=1) as proj,
            tc.tile_pool(name="dram", bufs=6, space="DRAM") as dpool,
            tc.tile_pool(name="adram", bufs=6, space="DRAM") as adpool,
        ):
            ident = consts.tile([P, P], BF16)
            from concourse.masks import make_identity
            make_identity(nc, ident[:])

            qbias = consts.tile([P, 2], F32)
            nc.sync.dma_start(qbias[:], qbias_in[:, :])
            bemb = consts.tile([P, D], F32)
            nc.sync.dma_start(bemb[:], _ap(bemb_in, 0, [[0, P], [1, D]]))

            # persistent data produced by projections
            qcT = proj.tile([P, L], F32R)   # (q*scale + rcb)^T, heads stacked
            qpT = proj.tile([P, L], F32R)   # (q*scale + rpb)^T
            kT = proj.tile([P, L], F32R)    # k^T, heads stacked
            vsb = proj.tile([P, NIT, HPC * V], BF16)   # v, j on partitions
            rkT = proj.tile([P, PE_PAD], F32R)         # rel_k^T, heads stacked

            # ---------------- phase D: rel_k ----------------
            with (
                tc.tile_pool(name="dw", bufs=1) as dw,
                tc.tile_pool(name="d_ps", bufs=2, space="PSUM") as d_ps,
            ):
                wrel = dw.tile([P, 2, P], F32R)
                pet = dw.tile([P, 2, PE_PAD], F32R)
                nc.scalar.dma_start(
                    wrel[:], _ap(wrel_in, 0, [[P, P], [P * P, 2], [1, P]])
                )
                nc.scalar.dma_start(
                    pet[:], _ap(pet_in, 0, [[PE_PAD, P], [P * PE_PAD, 2], [1, PE_PAD]])
                )
                for nj in range(6):
                    ps = d_ps.tile([P, 512], F32)
                    for kt in range(2):
                        nc.tensor.matmul(
                            ps[:],
                            wrel[:, kt, :],
                            pet[:, kt, nj * 512:(nj + 1) * 512],
                            start=(kt == 0), stop=(kt == 1),
                        )
                    nc.vector.tensor_copy(rkT[:, nj * 512:(nj + 1) * 512], ps[:])

            # ---------------- phases B/C: projections per L-chunk ----------------
            with (
                tc.tile_pool(name="w_qkv", bufs=1) as w_qkv,
                tc.tile_pool(name="xt_pool", bufs=2) as xt_pool,
                tc.tile_pool(name="b_ps", bufs=2, space="PSUM") as b_ps,
                tc.tile_pool(name="c_ps", bufs=3, space="PSUM") as c_ps,
            ):
                wqk = w_qkv.tile([P, NKT, 2 * P], F32R)
                wv = w_qkv.tile([P, NKT, HPC * V], F32R)
                nc.scalar.dma_start(
                    wqk[:], _ap(wqk_in, 0, [[2 * P, P], [P * 2 * P, NKT], [1, 2 * P]])
                )
                nc.scalar.dma_start(
                    wv[:],
                    _ap(wv_in, 0, [[HPC * V, P], [P * HPC * V, NKT], [1, HPC * V]]),
                )

                for lc in range(NCH):
                    xt = xt_pool.tile([P, NKT, CH], F32R, tag="xt")
                    # one DMA: x^T slice [D, CH] -> [128, 12, CH]
                    nc.sync.dma_start(
                        xt[:],
                        _ap(xt_in, lc * CH, [[L, P], [P * L, NKT], [1, CH]]),
                    )
                    # B: q/k projections for this chunk
                    for mi in range(2):
                        ps = b_ps.tile([P, CH], F32)
                        for kt in range(NKT):
                            nc.tensor.matmul(
                                ps[:],
                                wqk[:, kt, mi * P:(mi + 1) * P],
                                xt[:, kt, :],
                                start=(kt == 0), stop=(kt == NKT - 1),
                            )
                        sl = slice(lc * CH, (lc + 1) * CH)
                        if mi == 0:
                            nc.scalar.activation(
                                qcT[:, sl], ps[:], mybir.ActivationFunctionType.Identity,
                                bias=qbias[:, 0:1], scale=float(K) ** -0.5,
                            )
                            nc.scalar.activation(
                                qpT[:, sl], ps[:], mybir.ActivationFunctionType.Identity,
                                bias=qbias[:, 1:2], scale=float(K) ** -0.5,
                            )
                        else:
                            nc.vector.tensor_copy(kT[:, sl], ps[:])
                    # C: v projection for this chunk
                    for j4 in range(CH // P):
                        ps = c_ps.tile([P, HPC * V], F32)
                        for kt in range(NKT):
                            nc.tensor.matmul(
                                ps[:],
                                xt[:, kt, j4 * P:(j4 + 1) * P],
                                wv[:, kt, :],
                                start=(kt == 0), stop=(kt == NKT - 1),
                            )
                        nc.vector.tensor_copy(vsb[:, lc * (CH // P) + j4, :], ps[:])

            if DEBUG:
                nc.sync.dma_start(dbg_qcT[:, :], qcT[:].bitcast(F32))
                nc.sync.dma_start(dbg_kT[:, :], kT[:].bitcast(F32))
                nc.sync.dma_start(dbg_rkT[:, :], rkT[:].bitcast(F32))

            # ---------------- phase E: attention ----------------
            with (
                tc.tile_pool(name="attnT_p", bufs=2) as attnT_p,
                tc.tile_pool(name="outT_p", bufs=2) as outT_p,
                tc.tile_pool(name="sums_p", bufs=2) as sums_p,
                tc.tile_pool(name="band_sb_p", bufs=4) as band_sb_p,
                tc.tile_pool(name="rel_p", bufs=4) as rel_p,
                tc.tile_pool(name="attn_p", bufs=4) as attn_p,
                tc.tile_pool(name="band_ps", bufs=2, space="PSUM") as band_ps,
                tc.tile_pool(name="cont_ps", bufs=1, space="PSUM") as cont_ps,
                tc.tile_pool(name="o_ps", bufs=2, space="PSUM") as o_ps,
            ):
                for h in range(HPC):
                    hp = slice(h * K, (h + 1) * K)   # partition slice of this head
                    attnT = attnT_p.tile([P, NIT, L], BF16, tag="attnT")
                    sums = sums_p.tile([P, NIT], F32, tag="sums")

                    for it in range(NIT):
                        p0 = L - P - it * P   # band start: 1408 - 128*it
                        # rel band matmul -> bf16 band in sbuf
                        band_sb = band_sb_p.tile([P, BANDW], BF16, tag="band")
                        off = 0
                        for ci, cw in enumerate(BCH):
                            ps = band_ps.tile([P, 512], F32, tag="band_ps")
                            nc.tensor.matmul(
                                ps[:, :cw],
                                qpT[hp, it * P:(it + 1) * P],
                                rkT[hp, p0 + off:p0 + off + cw],
                                start=True, stop=True,
                            )
                            if ci % 2 == 0:
                                nc.vector.tensor_copy(
                                    band_sb[:, off:off + cw], ps[:, :cw]
                                )
                            else:
                                nc.scalar.copy(band_sb[:, off:off + cw], ps[:, :cw])
                            off += cw
                        band_dram = dpool.tile([P * BANDW], BF16, tag="band_dram")
                        # same HWDGE ring (ACT) as the skewed read below: ring
                        # FIFO guarantees the write drains before the read.
                        nc.scalar.dma_start(
                            band_dram.rearrange("(p w) -> p w", p=P), band_sb[:]
                        )

                        # content logits
                        pc = cont_ps.tile([P, L], F32, tag="cont")
                        for nj in range(NCH):
                            nc.tensor.matmul(
                                pc[:, nj * CH:(nj + 1) * CH],
                                qcT[hp, it * P:(it + 1) * P],
                                kT[hp, nj * CH:(nj + 1) * CH],
                                start=True, stop=False,
                            )

                        # shifted rel read-back: rel[p, j] = band[p, j + 127 - p]
                        rel_sb = rel_p.tile([P, L], BF16, tag="rel")
                        diag = _ap(
                            band_dram.tensor,
                            band_dram.offset + (P - 1),
                            [[BANDW - 1, P], [1, L]],
                        )
                        nc.scalar.dma_start(rel_sb[:], diag)
                        # add rel into pc on the PE: pc[:, c] += I @ rel[:, c]
                        for nj in range(NCH):
                            nc.tensor.matmul(
                                pc[:, nj * CH:(nj + 1) * CH],
                                ident[:],
                                rel_sb[:, nj * CH:(nj + 1) * CH],
                                start=False, stop=True,
                            )

                        # exp + row sums; bf16 attn
                        attn_sb = attn_p.tile([P, L], BF16, tag="attn")
                        nc.scalar.activation(
                            attn_sb[:], pc[:], mybir.ActivationFunctionType.Exp,
                            accum_out=sums[:, it:it + 1],
                        )
                        # attn -> DRAM -> batched transposed read into attnT
                        # (write + transposed read on the same SP HWDGE ring)
                        attn_dram = adpool.tile([P * L], BF16, tag="attn_dram")
                        nc.sync.dma_start(
                            attn_dram.rearrange("(p w) -> p w", p=P), attn_sb[:]
                        )
                        nc.sync.dma_start(
                            attnT[:, :, it * P:(it + 1) * P],
                            _ap(attn_dram.tensor, attn_dram.offset, [[L, P], [1, L]]),
                            transpose=True,
                        )
                        if DEBUG and h == 0 and it == 0:
                            nc.sync.dma_start(dbg_attn[:, :], attn_sb[:])

                    if DEBUG and h == 0:
                        nc.sync.dma_start(
                            dbg_attnT[:, :],
                            attnT.rearrange("p a b -> p (a b)"),
                        )

                    # reciprocal of row sums -> broadcast [M96, L] via DRAM
                    recip = sums_p.tile([P, NIT], F32, tag="recip")
                    nc.vector.reciprocal(recip[:], sums[:])
                    recip_dram = dpool.tile([L], F32, tag="recip_dram")
                    nc.scalar.dma_start(
                        recip_dram.rearrange("(it p) -> p it", p=P), recip[:]
                    )
                    recip_bc = sums_p.tile([M96, L], F32, tag="recip_bc")
                    nc.scalar.dma_start(
                        recip_bc[:],
                        _ap(recip_dram.tensor, recip_dram.offset, [[0, M96], [1, L]]),
                    )

                    # outT = v^T @ attnT, normalized; 2 tiles of 96 rows per head
                    for sub in range(2):
                        vs = h * V + sub * M96
                        outT_sb = outT_p.tile([M96, L], BF16, tag="outT_sb")
                        for ni in range(NCH):
                            po = o_ps.tile([P, CH], F32, tag="o_ps")
                            for jt in range(NIT):
                                nc.tensor.matmul(
                                    po[0:M96, :],
                                    vsb[:, jt, vs:vs + M96],
                                    attnT[:, jt, ni * CH:(ni + 1) * CH],
                                    start=(jt == 0), stop=(jt == NIT - 1),
                                )
                            nc.vector.tensor_tensor(
                                outT_sb[:, ni * CH:(ni + 1) * CH],
                                po[0:M96, :],
                                recip_bc[:, ni * CH:(ni + 1) * CH],
                                mybir.AluOpType.mult,
                            )
                        # store into this head's ccin: [vc 96, 1536 m]
                        vcb = sub * M96 * L
                        ccin_h = ccin1 if h == 0 else ccin2
                        nc.scalar.dma_start(
                            _ap(ccin_h, vcb, [[L, M96], [1, L]]),
                            outT_sb[:],
                        )
                        if DEBUG and h == 0 and sub == 0:
                            nc.sync.dma_start(dbg_outT[:, :], outT_sb[:])
                    if DEBUG and h == 0:
                        nc.sync.dma_start(dbg_sums[:, :], sums[:])

                    # AllGather this head's slab; head 0's collective runs
                    # while head 1's attention is still computing.
                    nc.gpsimd.collective_compute(
                        "AllGather",
                        mybir.AluOpType.bypass,
                        replica_groups=rg,
                        ins=[(ccin1 if h == 0 else ccin2)[:]],
                        outs=[(ccout1 if h == 0 else ccout2)[:]],
                    )

            with (
                tc.tile_pool(name="fin_sb", bufs=1) as fin_sb,
                tc.tile_pool(name="fout_p", bufs=3) as fout_p,
                tc.tile_pool(name="f_ps", bufs=2, space="PSUM") as f_ps,
            ):
                # full W_emb (bf16) for the post-AG projection: [128, 12, D];
                # loaded here so its SBUF does not occupy the attention phase
                # and the DMA overlaps the AllGather.
                wemb = fin_sb.tile([P, NKT, D], BF16)
                nc.scalar.dma_start(
                    wemb[:], _ap(wemb_in, 0, [[D, P], [P * D, NKT], [1, D]])
                )
                # keep the PE warm (HAM at 8/8) through the AllGather window;
                # results are never read.
                dummy_ps = f_ps.tile([P, CH], F32, tag="dummy_ps")
                for w in range(48):
                    nc.tensor.matmul(
                        dummy_ps[:],
                        qpT[0:K, 0:P],
                        rkT[0:K, (w % 6) * CH:(w % 6 + 1) * CH],
                        start=True, stop=True,
                    )

                # ccout{1,2}: [768 vc, 1536 m] each; read this core's m-slice
                # (dynamic offset coff = (c%4)*384 elements) -> [128, 6, 384]
                tmp = nc.sync.alloc_register("coff_reg")
                nc.sync.reg_load(tmp, coff_in[0:1, 0:1])
                coff = nc.sync.snap(tmp, donate=True, min_val=0, max_val=3 * LS)
                oall1 = fin_sb.tile([P, NKT // 2, LS], BF16)
                nc.sync.dma_start(
                    oall1[:], _ap(ccout1, coff, [[L, P], [P * L, NKT // 2], [1, LS]])
                )
                oall2 = fin_sb.tile([P, NKT // 2, LS], BF16)
                nc.sync.dma_start(
                    oall2[:], _ap(ccout2, coff, [[L, P], [P * L, NKT // 2], [1, LS]])
                )
                for mi in range(LS // P):
                    for nj in range(NCH):
                        pf = f_ps.tile([P, CH], F32, tag="f_ps")
                        for kt in range(NKT):
                            src = oall1 if kt < NKT // 2 else oall2
                            nc.tensor.matmul(
                                pf[:],
                                src[:, kt % (NKT // 2), mi * P:(mi + 1) * P],
                                wemb[:, kt, nj * CH:(nj + 1) * CH],
                                start=(kt == 0), stop=(kt == NKT - 1),
                            )
                        fo = fout_p.tile([P, CH], F32, tag="fout")
                        nc.vector.tensor_tensor(
                            fo[:], pf[:], bemb[:, nj * CH:(nj + 1) * CH],
                            mybir.AluOpType.add,
                        )
                        nc.sync.dma_start(
                            out_t[mi * P:(mi + 1) * P, nj * CH:(nj + 1) * CH], fo[:]
                        )

    nc.compile()
    return nc


_CACHE = {}


def _get_nc():
    if "nc" not in _CACHE:
        _CACHE["nc"] = _build_nc()
    return _CACHE["nc"]


def _make_in_maps(inputs, Wq, Wk, Wv, W_rel, W_emb, b_emb, rcb, rpb):
    import ml_dtypes

    pe = _positional_features()          # [3071, 192]
    pet = np.zeros((2 * P, PE_PAD), np.float32)
    pet[:F, :PE_LEN] = pe.T

    Wq_h = Wq.reshape(D, H, K)
    Wk_h = Wk.reshape(D, H, K)
    Wv_h = Wv.reshape(D, H, V)
    Wrel_h = W_rel.reshape(F, H, K)
    # rows reordered to match the two per-head AllGather outputs:
    # ccout1 = heads {0,2,4,6}, ccout2 = heads {1,3,5,7}
    wemb_perm = W_emb.reshape(H, V, D)[[0, 2, 4, 6, 1, 3, 5, 7]].reshape(H * V, D)
    wemb_bf = np.ascontiguousarray(wemb_perm).astype(ml_dtypes.bfloat16)

    in_maps = []
    for c in range(NCORES):
        b = c // 4
        g = c % 4
        h0, h1 = 2 * g, 2 * g + 1
        wqk = np.concatenate(
            [Wq_h[:, h0], Wq_h[:, h1], Wk_h[:, h0], Wk_h[:, h1]], axis=1
        )  # [D, 256]: Q stacked then K stacked
        wv2 = np.concatenate([Wv_h[:, h0], Wv_h[:, h1]], axis=1)  # [D, 384]
        wrel = np.zeros((2 * P, P), np.float32)
        wrel[:F, :K] = Wrel_h[:, h0]
        wrel[:F, K:] = Wrel_h[:, h1]
        qbias = np.stack(
            [np.concatenate([rcb[h0], rcb[h1]]), np.concatenate([rpb[h0], rpb[h1]])],
            axis=1,
        )  # [128, 2]
        in_maps.append({
            "xt": np.ascontiguousarray(inputs[b].T),
            "wqk": np.ascontiguousarray(wqk),
            "wv": np.ascontiguousarray(wv2),
            "wrel": wrel,
            "pet": pet,
            "wemb": wemb_bf,
            "qbias": np.ascontiguousarray(qbias),
            "bemb": b_emb.reshape(1, D),
            "coff": np.array([[g * LS]], dtype=np.uint32),
        })
    return in_maps


# ----------------------------------------------------------------------------
# entry point
# ----------------------------------------------------------------------------

def kernel(inputs, Wq, Wk, Wv, W_rel, W_emb, b_emb, rel_content_bias, rel_pos_bias):
    inputs = np.asarray(inputs, np.float32)
    Wq = np.asarray(Wq, np.float32)
    Wk = np.asarray(Wk, np.float32)
    Wv = np.asarray(Wv, np.float32)
    W_rel = np.asarray(W_rel, np.float32)
    W_emb = np.asarray(W_emb, np.float32)
    b_emb = np.asarray(b_emb, np.float32)
    rcb = np.asarray(rel_content_bias, np.float32).reshape(H, K)
    rpb = np.asarray(rel_pos_bias, np.float32).reshape(H, K)

    in_maps = _make_in_maps(inputs, Wq, Wk, Wv, W_rel, W_emb, b_emb, rcb, rpb)
    nc = _get_nc()
    res = run_bass_kernel_spmd(nc, in_maps, core_ids=list(range(NCORES)))

    out = np.empty((B, L, D), np.float32)
    for c in range(NCORES):
        b = c // 4
        g = c % 4
        out[b, g * LS:(g + 1) * LS, :] = res.results[c]["out"]
    return out
